# revision 4
# baseline (speedup 1.0000x reference)
"""Trainium2 Bass kernel for a convolutional GRU (nn_ConvolutionalRNN).

Reference semantics (per timestep t, torch-GRUCell-style with conv1d gates):
    gi = conv1d(x[t], w_ih) + b_ih          # [B, 3C, L], precomputable
    gh = conv1d(h,    w_hh) + b_hh          # [B, 3C, L], recurrent
    r = sigmoid(gi_r + gh_r); z = sigmoid(gi_z + gh_z)
    n = tanh(gi_n + r * gh_n)
    h = n + z * (h - n)  =  n*(1-z) + z*h
    ys[t] = h

Sharding: data-parallel over batch. B=16 across 8 NeuronCores -> 2 batch
items per core; weights replicated; T stays local (sequential recurrence).

Per-core layout: (batch, channel) = 128 on partitions, l on the free axis.
A single matmul computes a conv tap for BOTH batch items via block-diagonal
weights lhsT = diag(w, w) [128, 128] built on the host. The K=3 conv is 3
shifted matmuls accumulating in PSUM; the input-side conv accumulates into
the same banks, so gi never touches HBM. Matmuls run in float32r (fp32 data
rounded to 12-bit mantissa -> full-rate PE); all elementwise math is fp32.
The GRU update uses h_new = n*sigmoid(-pre_z) + z*h so only two vector ops
trail the tanh on the critical path.
"""

import numpy as np
from contextlib import ExitStack

from concourse import bacc, mybir
import concourse.tile as tile
from concourse.bass_utils import run_bass_kernel_spmd

T, B, CIN, COUT, L = 128, 16, 64, 64, 256
NCORES = 8
BL = B // NCORES          # batch per core = 2
P = BL * CIN              # 128 partitions = (b, c)
LP = L + 2                # padded length (zero border at l=0 and l=L+1)
F32 = mybir.dt.float32
F32R = mybir.dt.float32r
AF = mybir.ActivationFunctionType
ALU = mybir.AluOpType


def _round_fp32r(x: np.ndarray) -> np.ndarray:
    """Round fp32 to the fp32r grid (12-bit mantissa, round-nearest-even) —
    matches what TRN2 produces when an engine writes a float32r output."""
    u = np.ascontiguousarray(x, np.float32).view(np.uint32).copy()
    low = u & np.uint32(0xFFF)
    u &= np.uint32(0xFFFFF000)
    up = (low > 0x800) | ((low == 0x800) & (((u >> 12) & 1) == 1))
    u[up] += np.uint32(0x1000)
    return u.view(np.float32)


def _build_nc():
    nc = bacc.Bacc(trn_type="TRN2", target_bir_lowering=False, debug=False)

    # Per-core DRAM I/O ((b, c)-major is contiguous in the problem layout).
    x_d = nc.dram_tensor("x", [T, P, L], F32R, kind="ExternalInput").ap()
    h0_d = nc.dram_tensor("h0", [P, L], F32R, kind="ExternalInput").ap()
    # Block-diag weights: [src, chunk, tap, 128, 128]; src 0 = w_ih, 1 = w_hh;
    # chunk 0/1/2 = r/z/n gates.
    w_d = nc.dram_tensor("w", [2, 3, 3, P, P], F32R, kind="ExternalInput").ap()
    br_d = nc.dram_tensor("br", [P, 1], F32, kind="ExternalInput").ap()
    bz_d = nc.dram_tensor("bz", [P, 1], F32, kind="ExternalInput").ap()
    bzn_d = nc.dram_tensor("bzn", [P, 1], F32, kind="ExternalInput").ap()  # -b_z
    bihn_d = nc.dram_tensor("bihn", [P, 1], F32, kind="ExternalInput").ap()
    bhhn_d = nc.dram_tensor("bhhn", [P, 1], F32, kind="ExternalInput").ap()
    ys_d = nc.dram_tensor("ys", [T, P, L], F32, kind="ExternalOutput").ap()

    NB = 2  # x_buf double-buffer depth

    with tile.TileContext(nc) as tc, ExitStack() as ctx:
        persist = ctx.enter_context(tc.tile_pool(name="persist", bufs=1))
        work = ctx.enter_context(tc.tile_pool(name="work", bufs=3))
        psR = ctx.enter_context(tc.tile_pool(name="psR", bufs=2, space="PSUM"))
        psZ = ctx.enter_context(tc.tile_pool(name="psZ", bufs=2, space="PSUM"))
        psN1 = ctx.enter_context(tc.tile_pool(name="psN1", bufs=2, space="PSUM"))
        psN2 = ctx.enter_context(tc.tile_pool(name="psN2", bufs=2, space="PSUM"))

        # --- one-time setup -------------------------------------------------
        w = persist.tile([P, 2, 3, 3, P], F32)
        nc.sync.dma_start(out=w[:].bitcast(F32R),
                          in_=w_d.rearrange("s c k p q -> p s c k q"))
        biases = {}
        for name, d in (("br", br_d), ("bz", bz_d), ("bzn", bzn_d),
                        ("bihn", bihn_d), ("bhhn", bhhn_d)):
            bt = persist.tile([P, 1], F32, tag=name)
            nc.sync.dma_start(out=bt[:], in_=d)
            biases[name] = bt

        h_buf = persist.tile([P, LP], F32)
        x_buf = persist.tile([P, NB, LP], F32)
        nc.vector.memset(h_buf[:], 0.0)
        nc.vector.memset(x_buf[:], 0.0)
        nc.sync.dma_start(out=h_buf[:, 1:L + 1].bitcast(F32R), in_=h0_d)

        h_in = h_buf[:, 1:L + 1]               # [P, L] interior view

        def mm(out_ps, src, chunk, tap, rhs_ap, start, stop):
            nc.tensor.matmul(
                out_ps,
                w[:, src, chunk, tap, :].bitcast(F32R),
                rhs_ap.bitcast(F32R),
                start=start, stop=stop,
            )

        # --- the recurrence -------------------------------------------------
        for t in range(T):
            if t + 1 < T:
                xb_next = x_buf[:, (t + 1) % NB]
                nc.sync.dma_start(out=xb_next[:, 1:L + 1].bitcast(F32R),
                                  in_=x_d[t + 1])
            if t == 0:
                nc.sync.dma_start(out=x_buf[:, 0, 1:L + 1].bitcast(F32R),
                                  in_=x_d[0])
            xb = x_buf[:, t % NB]

            R = psR.tile([P, L], F32)
            Z = psZ.tile([P, L], F32)
            N1 = psN1.tile([P, L], F32)
            N2 = psN2.tile([P, L], F32)

            # input-side conv taps (only need x[t]; run ahead of the chain)
            for k in range(3):
                mm(R[:], 0, 0, k, xb[:, k:k + L], start=(k == 0), stop=False)
            for k in range(3):
                mm(Z[:], 0, 1, k, xb[:, k:k + L], start=(k == 0), stop=False)
            for k in range(3):
                mm(N1[:], 0, 2, k, xb[:, k:k + L], start=(k == 0), stop=(k == 2))
            # recurrent conv taps (critical path; need h(t-1))
            for k in range(3):
                mm(R[:], 1, 0, k, h_buf[:, k:k + L], start=False, stop=(k == 2))
            for k in range(3):
                mm(Z[:], 1, 1, k, h_buf[:, k:k + L], start=False, stop=(k == 2))
            for k in range(3):
                mm(N2[:], 1, 2, k, h_buf[:, k:k + L], start=(k == 0), stop=(k == 2))

            r = work.tile([P, L], F32, tag="r")
            z = work.tile([P, L], F32, tag="z")
            zc = work.tile([P, L], F32, tag="zc")
            nc.scalar.activation(r[:], R[:], AF.Sigmoid, bias=biases["br"][:])
            nc.scalar.activation(z[:], Z[:], AF.Sigmoid, bias=biases["bz"][:])
            nc.scalar.activation(zc[:], Z[:], AF.Sigmoid, bias=biases["bzn"][:],
                                 scale=-1.0)

            # n = tanh(i_n + r*(gh_n + b_hhn) + b_ihn)
            t1 = work.tile([P, L], F32, tag="t1")
            nc.vector.scalar_tensor_tensor(t1[:], N2[:], biases["bhhn"][:], r[:],
                                           op0=ALU.add, op1=ALU.mult)
            t2 = work.tile([P, L], F32, tag="t2")
            nc.vector.tensor_add(t2[:], t1[:], N1[:])
            n = work.tile([P, L], F32, tag="n")
            nc.scalar.activation(n[:], t2[:], AF.Tanh, bias=biases["bihn"][:])

            # h_new = z*h + n*(1-z)
            zh = work.tile([P, L], F32, tag="zh")
            nc.vector.tensor_mul(zh[:], z[:], h_in)
            nzc = work.tile([P, L], F32, tag="nzc")
            nc.vector.tensor_mul(nzc[:], n[:], zc[:])
            nc.vector.tensor_add(h_in.bitcast(F32R), zh[:], nzc[:])

            nc.sync.dma_start(out=ys_d[t], in_=h_in)

    nc.compile()
    return nc


_NC = None


def _get_nc():
    global _NC
    if _NC is None:
        _NC = _build_nc()
    return _NC


def _block_diag(wt: np.ndarray) -> np.ndarray:
    """wt: [3(chunk), 3(tap), CIN, 64]  ->  [3, 3, 128, 128] block-diagonal."""
    out = np.zeros((3, 3, P, P), np.float32)
    out[:, :, :CIN, :COUT] = wt
    out[:, :, CIN:, COUT:] = wt
    return out


def _prep_in_maps(x, h0, w_ih, w_hh, b_ih, b_hh):
    # [GATES, CIN, K] -> [3(chunk), K(tap), CIN, 64(gate)]
    def conv_w(wm):
        wm = np.asarray(wm, np.float32).reshape(3, COUT, CIN, 3)
        return np.transpose(wm, (0, 3, 2, 1))
    wblk = np.stack([_block_diag(conv_w(w_ih)), _block_diag(conv_w(w_hh))])
    wblk = _round_fp32r(wblk)                      # [2, 3, 3, 128, 128]

    b_ih = np.asarray(b_ih, np.float32)
    b_hh = np.asarray(b_hh, np.float32)
    def dup(v):                                    # [64] -> [128, 1]
        return np.concatenate([v, v]).reshape(P, 1).astype(np.float32)
    br = dup(b_ih[:COUT] + b_hh[:COUT])
    bz = dup(b_ih[COUT:2 * COUT] + b_hh[COUT:2 * COUT])
    biases = {"br": br, "bz": bz, "bzn": -bz,
              "bihn": dup(b_ih[2 * COUT:]), "bhhn": dup(b_hh[2 * COUT:])}

    x = _round_fp32r(np.asarray(x, np.float32))    # [T, B, CIN, L]
    h0 = _round_fp32r(np.asarray(h0, np.float32))  # [B, CIN... COUT, L]
    in_maps = []
    for c in range(NCORES):
        xs = np.ascontiguousarray(
            x[:, c * BL:(c + 1) * BL].reshape(T, P, L))
        h0s = np.ascontiguousarray(
            h0[c * BL:(c + 1) * BL].reshape(P, L))
        in_maps.append({"x": xs, "h0": h0s, "w": wblk, **biases})
    return in_maps


def kernel(x, h0, w_ih, w_hh, b_ih, b_hh):
    nc = _get_nc()
    in_maps = _prep_in_maps(x, h0, w_ih, w_hh, b_ih, b_hh)
    res = run_bass_kernel_spmd(nc, in_maps, list(range(NCORES)))
    ys = np.empty((T, B, COUT, L), np.float32)
    for c in range(NCORES):
        ys[:, c * BL:(c + 1) * BL] = res.results[c]["ys"].reshape(T, BL, COUT, L)
    return ys


# revision 5
# speedup vs baseline: 1.0129x; 1.0129x over previous
"""Trainium2 Bass kernel for a convolutional GRU (nn_ConvolutionalRNN).

Reference semantics (per timestep t, torch-GRUCell-style with conv1d gates):
    gi = conv1d(x[t], w_ih) + b_ih          # [B, 3C, L], precomputable
    gh = conv1d(h,    w_hh) + b_hh          # [B, 3C, L], recurrent
    r = sigmoid(gi_r + gh_r); z = sigmoid(gi_z + gh_z)
    n = tanh(gi_n + r * gh_n)
    h = n + z * (h - n)  =  z*h + n*(1-z)
    ys[t] = h

Sharding: data-parallel over batch. B=16 across 8 NeuronCores -> 2 batch
items per core; weights replicated; T stays local (sequential recurrence).

Per-core on-chip layout: channels on partitions, (b, l) on the free axis
(N=512 per matmul = one PSUM bank). The K=3 conv is 3 shifted matmuls
(contraction over CIN=64) accumulating in PSUM; the input-side conv is fused
into the same PSUM accumulation as the recurrent conv, so gi never touches
HBM. Matmuls run in float32r (fp32 data rounded to 12-bit mantissa ->
full-rate PE); everything else is fp32. The GRU update uses
h_new = z*h + n*sigmoid(-pre_z): z*h is computed off the critical path, so
only two vector ops trail the tanh.
"""

import numpy as np
from contextlib import ExitStack

from concourse import bacc, mybir
import concourse.tile as tile
from concourse.bass_utils import run_bass_kernel_spmd

T, B, CIN, COUT, L = 128, 16, 64, 64, 256
GATES = 3 * COUT
NCORES = 8
BL = B // NCORES          # batch per core = 2
LP = L + 2                # padded length (zero border at l=0 and l=L+1)
F32 = mybir.dt.float32
F32R = mybir.dt.float32r
AF = mybir.ActivationFunctionType
ALU = mybir.AluOpType


def _round_fp32r(x: np.ndarray) -> np.ndarray:
    """Round fp32 to the fp32r grid (12-bit mantissa, round-nearest-even) —
    matches what TRN2 produces when an engine writes a float32r output."""
    u = np.ascontiguousarray(x, np.float32).view(np.uint32).copy()
    low = u & np.uint32(0xFFF)
    u &= np.uint32(0xFFFFF000)
    up = (low > 0x800) | ((low == 0x800) & (((u >> 12) & 1) == 1))
    u[up] += np.uint32(0x1000)
    return u.view(np.float32)


def _build_nc():
    nc = bacc.Bacc(trn_type="TRN2", target_bir_lowering=False, debug=False)

    # Per-core DRAM I/O. Host pre-transposes to channel-major so every DMA
    # is 2 KB-contiguous per partition.
    x_d = nc.dram_tensor("x", [T, CIN, BL, L], F32R, kind="ExternalInput").ap()
    h0_d = nc.dram_tensor("h0", [COUT, BL, L], F32R, kind="ExternalInput").ap()
    wih_d = nc.dram_tensor("wih", [CIN, 3, GATES], F32R, kind="ExternalInput").ap()
    whh_d = nc.dram_tensor("whh", [CIN, 3, GATES], F32R, kind="ExternalInput").ap()
    brz_d = nc.dram_tensor("brz", [2 * COUT, 1], F32, kind="ExternalInput").ap()
    bzn_d = nc.dram_tensor("bzn", [COUT, 1], F32, kind="ExternalInput").ap()
    bihn_d = nc.dram_tensor("bihn", [COUT, 1], F32, kind="ExternalInput").ap()
    bhhn_d = nc.dram_tensor("bhhn", [COUT, 1], F32, kind="ExternalInput").ap()
    ys_d = nc.dram_tensor("ys", [T, COUT, BL, L], F32, kind="ExternalOutput").ap()

    NB = 2  # x_buf double-buffer depth

    with tile.TileContext(nc) as tc, ExitStack() as ctx:
        persist = ctx.enter_context(tc.tile_pool(name="persist", bufs=1))
        work = ctx.enter_context(tc.tile_pool(name="work", bufs=3))
        psA = ctx.enter_context(tc.tile_pool(name="psA", bufs=2, space="PSUM"))
        psB1 = ctx.enter_context(tc.tile_pool(name="psB1", bufs=2, space="PSUM"))
        psB2 = ctx.enter_context(tc.tile_pool(name="psB2", bufs=2, space="PSUM"))

        # --- one-time setup -------------------------------------------------
        wih = persist.tile([CIN, 3, GATES], F32)
        whh = persist.tile([CIN, 3, GATES], F32)
        nc.sync.dma_start(out=wih[:].bitcast(F32R), in_=wih_d)
        nc.sync.dma_start(out=whh[:].bitcast(F32R), in_=whh_d)
        brz = persist.tile([2 * COUT, 1], F32)
        bzn = persist.tile([COUT, 1], F32)
        bihn = persist.tile([COUT, 1], F32)
        bhhn = persist.tile([COUT, 1], F32)
        nc.sync.dma_start(out=brz[:], in_=brz_d)
        nc.sync.dma_start(out=bzn[:], in_=bzn_d)
        nc.sync.dma_start(out=bihn[:], in_=bihn_d)
        nc.sync.dma_start(out=bhhn[:], in_=bhhn_d)

        h_buf = persist.tile([COUT, BL, LP], F32)
        x_buf = persist.tile([CIN, NB, BL, LP], F32)
        nc.vector.memset(h_buf[:], 0.0)
        nc.vector.memset(x_buf[:], 0.0)
        nc.sync.dma_start(out=h_buf[:, :, 1:L + 1].bitcast(F32R), in_=h0_d)

        h_in = h_buf[:, :, 1:L + 1]            # [COUT, BL, L] interior view

        def mm(out_ps, w_tile, g0, g1, rhs_buf, tap, start, stop):
            """out_ps += w[:, tap, g0:g1]^T @ rhs_buf shifted by tap."""
            nc.tensor.matmul(
                out_ps,
                w_tile[:, tap, g0:g1].bitcast(F32R),
                rhs_buf[:, :, tap:tap + L].bitcast(F32R),
                start=start, stop=stop,
            )

        # --- the recurrence -------------------------------------------------
        for t in range(T):
            xb = x_buf[:, t % NB]              # [CIN, BL, LP]
            nc.sync.dma_start(out=xb[:, :, 1:L + 1].bitcast(F32R), in_=x_d[t])

            pre_rz = psA.tile([2 * COUT, BL, L], F32)   # i_r+h_r | i_z+h_z
            i_n = psB1.tile([COUT, BL, L], F32)
            gh_n = psB2.tile([COUT, BL, L], F32)

            # input-side convs (off critical path; only need x[t])
            for k in range(3):
                mm(pre_rz[:], wih, 0, 128, xb, k, start=(k == 0), stop=False)
            for k in range(3):
                mm(i_n[:], wih, 128, 192, xb, k, start=(k == 0), stop=(k == 2))
            # recurrent convs (critical path; need h(t-1))
            for k in range(3):
                mm(pre_rz[:], whh, 0, 128, h_buf, k, start=False, stop=(k == 2))
            for k in range(3):
                mm(gh_n[:], whh, 128, 192, h_buf, k, start=(k == 0), stop=(k == 2))

            r = work.tile([COUT, BL, L], F32, tag="r")
            z = work.tile([COUT, BL, L], F32, tag="z")
            zc = work.tile([COUT, BL, L], F32, tag="zc")
            nc.scalar.activation(r[:], pre_rz[0:COUT], AF.Sigmoid, bias=brz[0:COUT])
            nc.scalar.activation(z[:], pre_rz[COUT:2 * COUT], AF.Sigmoid,
                                 bias=brz[COUT:2 * COUT])
            nc.scalar.activation(zc[:], pre_rz[COUT:2 * COUT], AF.Sigmoid,
                                 bias=bzn[:], scale=-1.0)

            # t1 = (gh_n + b_hhn) * r ; t2 = t1 + i_n ; n = tanh(t2 + b_ihn)
            t1 = work.tile([COUT, BL, L], F32, tag="t1")
            nc.vector.scalar_tensor_tensor(t1[:], gh_n[:], bhhn[:], r[:],
                                           op0=ALU.add, op1=ALU.mult)
            t2 = work.tile([COUT, BL, L], F32, tag="t2")
            nc.vector.tensor_add(t2[:], t1[:], i_n[:])
            n = work.tile([COUT, BL, L], F32, tag="n")
            nc.scalar.activation(n[:], t2[:], AF.Tanh, bias=bihn[:])

            # h_new = z*h + n*zc   (z*h runs before tanh finishes)
            zh = work.tile([COUT, BL, L], F32, tag="zh")
            nc.vector.tensor_mul(zh[:], z[:], h_in)
            nzc = work.tile([COUT, BL, L], F32, tag="nzc")
            nc.vector.tensor_mul(nzc[:], n[:], zc[:])
            nc.vector.tensor_add(h_in.bitcast(F32R), zh[:], nzc[:])

            nc.sync.dma_start(out=ys_d[t], in_=h_in)

    nc.compile()
    return nc


_NC = None


def _get_nc():
    global _NC
    if _NC is None:
        _NC = _build_nc()
    return _NC


def _prep_in_maps(x, h0, w_ih, w_hh, b_ih, b_hh):
    # weights: [GATES, CIN, K] -> [CIN, K, GATES], fp32r-rounded
    wih_t = _round_fp32r(np.transpose(np.asarray(w_ih, np.float32), (1, 2, 0)))
    whh_t = _round_fp32r(np.transpose(np.asarray(w_hh, np.float32), (1, 2, 0)))
    b_ih = np.asarray(b_ih, np.float32)
    b_hh = np.asarray(b_hh, np.float32)
    brz = (b_ih[:2 * COUT] + b_hh[:2 * COUT]).reshape(2 * COUT, 1)
    bzn = -brz[COUT:2 * COUT]
    bihn = b_ih[2 * COUT:].reshape(COUT, 1)
    bhhn = b_hh[2 * COUT:].reshape(COUT, 1)

    x = _round_fp32r(np.asarray(x, np.float32))
    h0 = _round_fp32r(np.asarray(h0, np.float32))
    in_maps = []
    for c in range(NCORES):
        xs = np.ascontiguousarray(
            np.transpose(x[:, c * BL:(c + 1) * BL], (0, 2, 1, 3)))
        h0s = np.ascontiguousarray(
            np.transpose(h0[c * BL:(c + 1) * BL], (1, 0, 2)))
        in_maps.append({
            "x": xs, "h0": h0s, "wih": wih_t, "whh": whh_t,
            "brz": brz, "bzn": bzn, "bihn": bihn, "bhhn": bhhn,
        })
    return in_maps


def kernel(x, h0, w_ih, w_hh, b_ih, b_hh):
    nc = _get_nc()
    in_maps = _prep_in_maps(x, h0, w_ih, w_hh, b_ih, b_hh)
    res = run_bass_kernel_spmd(nc, in_maps, list(range(NCORES)))
    ys = np.empty((T, B, COUT, L), np.float32)
    for c in range(NCORES):
        ys[:, c * BL:(c + 1) * BL] = np.transpose(
            res.results[c]["ys"], (0, 2, 1, 3))
    return ys
